# revision 5
# baseline (speedup 1.0000x reference)
"""Distributed Trainium2 kernel for nn_AttentionComm (gnn message passing).

Sharding: agent dimension N=64 split across 8 NeuronCores (8 agents/core).
Each core streams its 24 MB of per-agent fp32 projection weights (the
memory-bound part) through a casting DMA (fp32 in HBM -> bf16 in SBUF so the
TensorEngine moving operand runs at full rate), computes Q/K/V locally,
all-gathers the small K,V (64 x 512), runs the complete-graph attention for
its 8 receivers, and computes its partial infer-loss. Host concatenates
messages and sums the loss partials.
"""

import numpy as np
import ml_dtypes

from concourse import bacc, mybir, tile
from concourse.bass_utils import run_bass_kernel_spmd
from concourse.masks import make_identity

N = 64          # agents
Z = 512         # z_dim
D = 512         # d_model
C = 8           # cores
L = N // C      # agents per core = 8
KC = D // 128   # contraction chunks = 4
F32 = mybir.dt.float32
BF16 = mybir.dt.bfloat16
NPBF16 = ml_dtypes.bfloat16
NEG = -1.0e4    # masked-logit value; exp underflows to exactly 0.0 in f32

_cache = {}


def _build():
    nc = bacc.Bacc(
        "TRN2",
        target_bir_lowering=False,
        debug=False,
        enable_asserts=False,
        num_devices=C,
    )

    # Per-core external inputs (host pre-layouts; all DMA-contiguous):
    # w* : fp32 [128, L*KC*512] ; [p, n*2048 + k*512 + d] = W[n, d, k*128+p]
    wq = nc.dram_tensor("wq", [128, L * KC * 512], F32, kind="ExternalInput")
    wk = nc.dram_tensor("wk", [128, L * KC * 512], F32, kind="ExternalInput")
    wv = nc.dram_tensor("wv", [128, L * KC * 512], F32, kind="ExternalInput")
    # z expansions (bf16): [128, KC*L*L]; [p, k*64+n*8+c] = z[n, k*128+p]*(c==n)
    zq = nc.dram_tensor("zq", [128, KC * L * L], BF16, kind="ExternalInput")
    ze = nc.dram_tensor("ze", [128, KC * L * L], BF16, kind="ExternalInput")
    # infer head (bf16), transposed+chunked: [p, k*512+zz] = Wi[zz, k*128+p]
    wi = nc.dram_tensor("wi", [128, KC * Z], BF16, kind="ExternalInput")
    # maskT[j, i] = NEG where i == global_id(j) else 0
    maskT = nc.dram_tensor("maskT", [L, N], F32, kind="ExternalInput")
    # bmz[j, zz] = bi[zz] - z_local[j, zz]
    bmz = nc.dram_tensor("bmz", [L, Z], F32, kind="ExternalInput")

    out_msg = nc.dram_tensor("out_msg", [L, D], F32, kind="ExternalOutput")
    out_sse = nc.dram_tensor("out_sse", [L, 1], F32, kind="ExternalOutput")

    # collective bounce buffers (internal DRAM)
    k_in = nc.dram_tensor("k_in", [L, D], F32)
    v_in = nc.dram_tensor("v_in", [L, D], F32)
    k_out = nc.dram_tensor("k_out", [N, D], F32, addr_space="Shared")
    v_out = nc.dram_tensor("v_out", [N, D], F32, addr_space="Shared")

    with tile.TileContext(nc) as tc:
        with (
            tc.tile_pool(name="consts", bufs=1) as consts,
            tc.tile_pool(name="wpool", bufs=3) as wpool,
            tc.tile_pool(name="spool", bufs=1) as spool,
            tc.tile_pool(name="psum_acc", bufs=2, space="PSUM") as psum_acc,
            tc.tile_pool(name="psum_tr", bufs=2, space="PSUM") as psum_tr,
            tc.tile_pool(name="psum_s", bufs=1, space="PSUM") as psum_s,
        ):
            # --- small constants (HWDGE; weights go on the SWDGE ring) ---
            identity = consts.tile([128, 128], F32)
            make_identity(nc, identity)
            zq_sb = consts.tile([128, KC * L * L], BF16)
            nc.sync.dma_start(out=zq_sb[:, :], in_=zq[:, :])
            ze_sb = consts.tile([128, KC * L * L], BF16)
            nc.sync.dma_start(out=ze_sb[:, :], in_=ze[:, :])
            wi_sb = consts.tile([128, KC * Z], BF16)
            nc.sync.dma_start(out=wi_sb[:, :], in_=wi[:, :])
            mask_sb = consts.tile([L, N], F32)
            nc.sync.dma_start(out=mask_sb[:, :], in_=maskT[:, :])
            bmz_sb = consts.tile([L, Z], F32)
            nc.sync.dma_start(out=bmz_sb[:, :], in_=bmz[:, :])

            # --- Q/K/V per-agent GEMVs, streaming weights (order: K, V, Q) ---
            HALF = L // 2  # agents per DMA chunk (4 MB fp32 -> 2 MB bf16)

            def gemv_all(wdram, zsb):
                """psum [L, D] f32: row n = W[n] @ z[n] (block-one-hot lhsT)."""
                ps = psum_acc.tile([L, D], F32, tag="acc")
                for h in range(2):
                    wt = wpool.tile([128, HALF * KC * 512], BF16, tag="w")
                    nc.gpsimd.dma_start(  # SWDGE: casts fp32 -> bf16 inline
                        out=wt[:, :],
                        in_=wdram[:, h * HALF * KC * 512:(h + 1) * HALF * KC * 512],
                    )
                    for nl in range(HALF):
                        n = h * HALF + nl
                        for k in range(KC):
                            nc.tensor.matmul(
                                ps[:, :],
                                zsb[:, k * (L * L) + n * L: k * (L * L) + n * L + L],
                                wt[:, (nl * KC + k) * 512:(nl * KC + k) * 512 + 512],
                                start=(h == 0 and nl == 0 and k == 0),
                                stop=(h == 1 and nl == HALF - 1 and k == KC - 1),
                            )
                return ps

            k_ps = gemv_all(wk, ze_sb)
            k_sb = spool.tile([L, D], F32)
            nc.vector.tensor_copy(k_sb[:, :], k_ps[:, :])
            nc.sync.dma_start(out=k_in[:, :], in_=k_sb[:, :])
            nc.gpsimd.collective_compute(
                "AllGather",
                mybir.AluOpType.bypass,
                replica_groups=[list(range(C))],
                ins=[k_in[:, :]],
                outs=[k_out[:, :]],
            )

            v_ps = gemv_all(wv, ze_sb)
            v_sb = spool.tile([L, D], F32)
            nc.vector.tensor_copy(v_sb[:, :], v_ps[:, :])
            nc.sync.dma_start(out=v_in[:, :], in_=v_sb[:, :])
            nc.gpsimd.collective_compute(
                "AllGather",
                mybir.AluOpType.bypass,
                replica_groups=[list(range(C))],
                ins=[v_in[:, :]],
                outs=[v_out[:, :]],
            )

            q_ps = gemv_all(wq, zq_sb)
            q_sb = spool.tile([L, D], F32)
            nc.vector.tensor_copy(q_sb[:, :], q_ps[:, :])

            # --- infer-loss (local; only needs V_local, runs under Wq stream) ---
            vt_sb = spool.tile([128, KC * L], BF16)
            for k in range(KC):
                tp = psum_tr.tile([128, N], F32, tag="tr")
                nc.tensor.transpose(
                    tp[:, 0:L], v_sb[:, k * 128:(k + 1) * 128], identity[0:L, 0:L]
                )
                nc.vector.tensor_copy(vt_sb[:, k * L:(k + 1) * L], tp[:, 0:L])
            hat_ps = psum_acc.tile([L, Z], F32, tag="acc")
            for k in range(KC):
                nc.tensor.matmul(
                    hat_ps[:, :],
                    vt_sb[:, k * L:(k + 1) * L],
                    wi_sb[:, k * 512:(k + 1) * 512],
                    start=(k == 0),
                    stop=(k == KC - 1),
                )
            diff_sb = spool.tile([L, Z], F32)
            nc.vector.tensor_add(diff_sb[:, :], hat_ps[:, :], bmz_sb[:, :])
            d2_sb = spool.tile([L, Z], F32)
            sse_sb = spool.tile([L, 1], F32)
            nc.scalar.activation(
                d2_sb[:, :],
                diff_sb[:, :],
                mybir.ActivationFunctionType.Square,
                accum_out=sse_sb[:, :],
            )
            nc.scalar.dma_start(out=out_sse[:, :], in_=sse_sb[:, :])

            # --- load gathered K,V; build transposed bf16 views ---
            kf_sb = spool.tile([N, D], F32)
            nc.sync.dma_start(out=kf_sb[:, :], in_=k_out[:, :])
            vf_f32 = spool.tile([N, D], F32)
            nc.sync.dma_start(out=vf_f32[:, :], in_=v_out[:, :])
            vf_sb = spool.tile([N, D], BF16)
            nc.vector.tensor_copy(vf_sb[:, :], vf_f32[:, :])
            qt_sb = spool.tile([128, KC * L], BF16)
            for k in range(KC):
                tp = psum_tr.tile([128, N], F32, tag="tr")
                nc.tensor.transpose(
                    tp[:, 0:L], q_sb[:, k * 128:(k + 1) * 128], identity[0:L, 0:L]
                )
                nc.vector.tensor_copy(qt_sb[:, k * L:(k + 1) * L], tp[:, 0:L])
            kt_sb = spool.tile([128, KC * N], BF16)
            for k in range(KC):
                tp = psum_tr.tile([128, N], F32, tag="tr")
                nc.tensor.transpose(
                    tp[:, :], kf_sb[:, k * 128:(k + 1) * 128], identity[0:N, 0:N]
                )
                nc.vector.tensor_copy(kt_sb[:, k * N:(k + 1) * N], tp[:, :])

            # --- S_T[j, i] = <Q_j, K_i> (pre-scaled via zq); mask; softmax ---
            s_ps = psum_s.tile([L, N], F32)
            for k in range(KC):
                nc.tensor.matmul(
                    s_ps[:, :],
                    qt_sb[:, k * L:(k + 1) * L],
                    kt_sb[:, k * N:(k + 1) * N],
                    start=(k == 0),
                    stop=(k == KC - 1),
                )
            s_sb = spool.tile([L, N], F32)
            nc.vector.tensor_add(s_sb[:, :], s_ps[:, :], mask_sb[:, :])
            nmax_sb = spool.tile([L, 1], F32)
            nc.vector.tensor_reduce(
                nmax_sb[:, :], s_sb[:, :], mybir.AxisListType.X,
                mybir.AluOpType.max, negate=True,
            )
            p_sb = spool.tile([L, N], F32)
            den_sb = spool.tile([L, 1], F32)
            nc.scalar.activation(
                p_sb[:, :],
                s_sb[:, :],
                mybir.ActivationFunctionType.Exp,
                bias=nmax_sb[:, 0:1],
                scale=1.0,
                accum_out=den_sb[:, :],
            )
            rden_sb = spool.tile([L, 1], F32)
            nc.vector.reciprocal(rden_sb[:, :], den_sb[:, :])
            nc.vector.tensor_scalar_mul(p_sb[:, :], p_sb[:, :], rden_sb[:, 0:1])

            # --- messages[j] = sum_i alpha[i, j] * V[i] ---
            a_ps = psum_tr.tile([N, L], F32, tag="al")
            nc.tensor.transpose(a_ps[:, :], p_sb[:, :], identity[0:L, 0:L])
            a_sb = spool.tile([N, L], BF16)
            nc.vector.tensor_copy(a_sb[:, :], a_ps[:, :])
            m_ps = psum_acc.tile([L, D], F32, tag="acc")
            nc.tensor.matmul(m_ps[:, :], a_sb[:, :], vf_sb[:, :], start=True, stop=True)
            msg_sb = spool.tile([L, D], F32)
            nc.vector.tensor_copy(msg_sb[:, :], m_ps[:, :])
            nc.scalar.dma_start(out=out_msg[:, :], in_=msg_sb[:, :])

    nc.compile()
    return nc


def _prep_inputs(z, Wq, Wk, Wv, Wi, bi):
    """Host-side shard + relayout. Returns in_maps for the 8 cores."""
    z = np.asarray(z, np.float32)
    Wq = np.asarray(Wq, np.float32)
    Wk = np.asarray(Wk, np.float32)
    Wv = np.asarray(Wv, np.float32)
    Wi = np.asarray(Wi, np.float32)
    bi = np.asarray(bi, np.float32)
    scale = np.float32(1.0 / np.sqrt(np.float32(D)))

    # Wi.T chunked (bf16): [p, k*512 + zz] = Wi[zz, k*128+p]
    wi_h = np.ascontiguousarray(
        Wi.T.reshape(KC, 128, Z).transpose(1, 0, 2).reshape(128, KC * Z)
    ).astype(NPBF16)

    def wlayout(Wl):
        # [p, n*2048 + k*512 + d] = Wl[n, d, k*128+p]
        return np.ascontiguousarray(
            Wl.reshape(L, D, KC, 128).transpose(3, 0, 2, 1).reshape(128, L * KC * D)
        )

    def zexpand(zl):
        # [p, k*64 + n*8 + c] = zl[n, k*128+p] * (c == n)
        ze = np.zeros((128, KC, L, L), np.float32)
        for n in range(L):
            ze[:, :, n, n] = zl[n].reshape(KC, 128).T
        return np.ascontiguousarray(ze.reshape(128, KC * L * L)).astype(NPBF16)

    in_maps = []
    for c in range(C):
        sl = slice(c * L, (c + 1) * L)
        zl = z[sl]
        maskT = np.zeros((L, N), np.float32)
        for j in range(L):
            maskT[j, c * L + j] = NEG
        in_maps.append({
            "wq": wlayout(Wq[sl]),
            "wk": wlayout(Wk[sl]),
            "wv": wlayout(Wv[sl]),
            "zq": zexpand(zl * scale),
            "ze": zexpand(zl),
            "wi": wi_h,
            "maskT": maskT,
            "bmz": np.ascontiguousarray(bi[None, :] - zl),
        })
    return in_maps


def run(z, Wq, Wk, Wv, Wi, bi, trace=False, **trace_kwargs):
    if "nc" not in _cache:
        _cache["nc"] = _build()
    nc = _cache["nc"]
    in_maps = _prep_inputs(z, Wq, Wk, Wv, Wi, bi)
    res = run_bass_kernel_spmd(
        nc, in_maps, core_ids=list(range(C)), trace=trace, **trace_kwargs
    )
    msgs = np.concatenate([r["out_msg"] for r in res.results], axis=0)
    sse = np.concatenate([r["out_sse"] for r in res.results], axis=0)
    loss = np.float32(sse.sum() / (N * Z))
    return (msgs, loss), res


def kernel(z, Wq, Wk, Wv, Wi, bi):
    (msgs, loss), _ = run(z, Wq, Wk, Wv, Wi, bi)
    return msgs, loss


# revision 12
# speedup vs baseline: 1.0850x; 1.0850x over previous
"""Distributed Trainium2 kernel for nn_AttentionComm (gnn message passing).

Sharding: agent dimension N=64 split across 8 NeuronCores (8 agents/core).
Each core streams its 24 MB of per-agent fp32 projection weights (the
memory-bound part) over both HWDGE rings, casts each chunk to bf16 on the
VectorEngine (so the TensorEngine moving operand runs at full rate), computes
Q/K/V locally, all-gathers the small K,V (64 x 512) during the stream, runs
the complete-graph attention for its 8 receivers, and computes its partial
infer-loss. Host concatenates messages and sums the loss partials.
"""

import numpy as np
import ml_dtypes

from concourse import bacc, mybir, tile
from concourse.bass_utils import run_bass_kernel_spmd
from concourse.masks import make_identity

N = 64          # agents
Z = 512         # z_dim
D = 512         # d_model
C = 8           # cores
L = N // C      # agents per core = 8
KC = D // 128   # contraction chunks = 4
AW = KC * 512   # free-axis columns per agent in the weight layout = 2048
F32 = mybir.dt.float32
BF16 = mybir.dt.bfloat16
NPBF16 = ml_dtypes.bfloat16
NEG = -1.0e4    # masked-logit value; exp underflows to exactly 0.0 in f32

_cache = {}


def _build():
    nc = bacc.Bacc(
        "TRN2",
        target_bir_lowering=False,
        debug=False,
        enable_asserts=False,
        num_devices=C,
    )

    # Per-core external inputs (host pre-layouts; all DMA-contiguous):
    # w* : fp32 [128, L*2048] ; [p, n*2048 + k*512 + d] = W[n, d, k*128+p]
    wq = nc.dram_tensor("wq", [128, L * AW], F32, kind="ExternalInput")
    wk = nc.dram_tensor("wk", [128, L * AW], F32, kind="ExternalInput")
    wv = nc.dram_tensor("wv", [128, L * AW], F32, kind="ExternalInput")
    # z expansions (bf16): [128, KC*L*L]; [p, k*64+n*8+c] = z[n, k*128+p]*(c==n)
    zq = nc.dram_tensor("zq", [128, KC * L * L], BF16, kind="ExternalInput")
    ze = nc.dram_tensor("ze", [128, KC * L * L], BF16, kind="ExternalInput")
    # infer head (bf16), transposed+chunked: [p, k*512+zz] = Wi[zz, k*128+p]
    wi = nc.dram_tensor("wi", [128, KC * Z], BF16, kind="ExternalInput")
    # maskT[j, i] = NEG where i == global_id(j) else 0
    maskT = nc.dram_tensor("maskT", [L, N], F32, kind="ExternalInput")
    # bmz[j, zz] = bi[zz] - z_local[j, zz]
    bmz = nc.dram_tensor("bmz", [L, Z], F32, kind="ExternalInput")

    out_msg = nc.dram_tensor("out_msg", [L, D], F32, kind="ExternalOutput")
    out_sse = nc.dram_tensor("out_sse", [L, 1], F32, kind="ExternalOutput")

    # collective bounce buffers (internal DRAM)
    k_in = nc.dram_tensor("k_in", [L, D], F32)
    v_in = nc.dram_tensor("v_in", [L, D], F32)
    k_out = nc.dram_tensor("k_out", [N, D], F32, addr_space="Shared")
    v_out = nc.dram_tensor("v_out", [N, D], F32, addr_space="Shared")
    warm_in = nc.dram_tensor("warm_in", [1, 128], F32)
    warm_out = nc.dram_tensor("warm_out", [C, 128], F32, addr_space="Shared")

    ring = [nc.sync, nc.scalar]   # two HWDGE rings, alternate per chunk
    ring_i = [0]

    with tile.TileContext(nc) as tc:
        with (
            tc.tile_pool(name="consts", bufs=1) as consts,
            tc.tile_pool(name="wf", bufs=4) as wf_pool,
            tc.tile_pool(name="wb", bufs=4) as wb_pool,
            tc.tile_pool(name="spool", bufs=1) as spool,
            tc.tile_pool(name="psum_acc", bufs=2, space="PSUM") as psum_acc,
            tc.tile_pool(name="psum_q", bufs=2, space="PSUM") as psum_q,
            tc.tile_pool(name="psum_tr", bufs=2, space="PSUM") as psum_tr,
            tc.tile_pool(name="psum_s", bufs=1, space="PSUM") as psum_s,
        ):
            # --- warmup collective: absorbs the ncfw entry barrier while the
            # weight stream runs; no data dependencies so it fires immediately
            nc.gpsimd.collective_compute(
                "AllGather",
                mybir.AluOpType.bypass,
                replica_groups=[list(range(C))],
                ins=[warm_in[:, :]],
                outs=[warm_out[:, :]],
            )

            # --- small constants (SWDGE ring; weights own the HWDGE rings) ---
            identity = consts.tile([128, 128], F32)
            make_identity(nc, identity)
            zq_sb = consts.tile([128, KC * L * L], BF16)
            nc.gpsimd.dma_start(out=zq_sb[:, :], in_=zq[:, :])
            ze_sb = consts.tile([128, KC * L * L], BF16)
            nc.gpsimd.dma_start(out=ze_sb[:, :], in_=ze[:, :])
            wi_sb = consts.tile([128, KC * Z], BF16)
            nc.gpsimd.dma_start(out=wi_sb[:, :], in_=wi[:, :])
            mask_sb = consts.tile([L, N], F32)
            nc.gpsimd.dma_start(out=mask_sb[:, :], in_=maskT[:, :])
            bmz_sb = consts.tile([L, Z], F32)
            nc.gpsimd.dma_start(out=bmz_sb[:, :], in_=bmz[:, :])

            def stream_chunk(wdram, a0, na):
                """DMA fp32 agents [a0, a0+na) and DVE-cast to bf16."""
                wtf = wf_pool.tile([128, 2 * AW], F32, tag="wf")
                eng = ring[ring_i[0] % 2]
                ring_i[0] += 1
                eng.dma_start(
                    out=wtf[:, :na * AW], in_=wdram[:, a0 * AW:(a0 + na) * AW]
                )
                wtb = wb_pool.tile([128, 2 * AW], BF16, tag="wb")
                nc.vector.tensor_copy(wtb[:, :na * AW], wtf[:, :na * AW])
                return wtb

            def gemv_kv(wdram):
                """psum [L, D] f32: row n = W[n] @ z[n] (block-one-hot lhsT)."""
                ps = psum_acc.tile([L, D], F32, tag="acc")
                for h in range(4):           # 4 chunks x 2 agents (2 MB fp32)
                    wtb = stream_chunk(wdram, 2 * h, 2)
                    for nl in range(2):
                        n = 2 * h + nl
                        for k in range(KC):
                            nc.tensor.matmul(
                                ps[:, :],
                                ze_sb[:, k * (L * L) + n * L: k * (L * L) + n * L + L],
                                wtb[:, (nl * KC + k) * 512:(nl * KC + k) * 512 + 512],
                                start=(h == 0 and nl == 0 and k == 0),
                                stop=(h == 3 and nl == 1 and k == KC - 1),
                            )
                return ps

            # --- K then V (AllGathers overlap the rest of the stream) ---
            k_ps = gemv_kv(wk)
            k_sb = spool.tile([L, D], F32)
            nc.vector.tensor_copy(k_sb[:, :], k_ps[:, :])
            nc.gpsimd.dma_start(out=k_in[:, :], in_=k_sb[:, :])
            nc.gpsimd.collective_compute(
                "AllGather",
                mybir.AluOpType.bypass,
                replica_groups=[list(range(C))],
                ins=[k_in[:, :]],
                outs=[k_out[:, :]],
            )

            v_ps = gemv_kv(wv)
            v_sb = spool.tile([L, D], F32)
            nc.vector.tensor_copy(v_sb[:, :], v_ps[:, :])
            nc.gpsimd.dma_start(out=v_in[:, :], in_=v_sb[:, :])
            nc.gpsimd.collective_compute(
                "AllGather",
                mybir.AluOpType.bypass,
                replica_groups=[list(range(C))],
                ins=[v_in[:, :]],
                outs=[v_out[:, :]],
            )

            # --- infer-loss (local; only needs V_local) ---
            vt_sb = spool.tile([128, KC * L], BF16)
            for k in range(KC):
                tp = psum_tr.tile([128, N], F32, tag="tr")
                nc.tensor.transpose(
                    tp[:, 0:L], v_sb[:, k * 128:(k + 1) * 128], identity[0:L, 0:L]
                )
                nc.vector.tensor_copy(vt_sb[:, k * L:(k + 1) * L], tp[:, 0:L])
            hat_ps = psum_acc.tile([L, Z], F32, tag="acc")
            for k in range(KC):
                nc.tensor.matmul(
                    hat_ps[:, :],
                    vt_sb[:, k * L:(k + 1) * L],
                    wi_sb[:, k * 512:(k + 1) * 512],
                    start=(k == 0),
                    stop=(k == KC - 1),
                )
            diff_sb = spool.tile([L, Z], F32)
            nc.vector.tensor_add(diff_sb[:, :], hat_ps[:, :], bmz_sb[:, :])
            d2_sb = spool.tile([L, Z], F32)
            sse_sb = spool.tile([L, 1], F32)
            nc.scalar.activation(
                d2_sb[:, :],
                diff_sb[:, :],
                mybir.ActivationFunctionType.Square,
                accum_out=sse_sb[:, :],
            )
            nc.scalar.dma_start(out=out_sse[:, :], in_=sse_sb[:, :])

            # --- load gathered K,V on the (soon idle) HWDGE rings ---
            kf_sb = spool.tile([N, D], F32)
            nc.sync.dma_start(out=kf_sb[:, :], in_=k_out[:, :])
            vf_f32 = spool.tile([N, D], F32)
            nc.scalar.dma_start(out=vf_f32[:, :], in_=v_out[:, :])
            vf_sb = spool.tile([N, D], BF16)
            nc.vector.tensor_copy(vf_sb[:, :], vf_f32[:, :])
            kt_sb = spool.tile([128, KC * N], BF16)
            for k in range(KC):
                tp = psum_tr.tile([128, N], F32, tag="tr")
                nc.tensor.transpose(
                    tp[:, :], kf_sb[:, k * 128:(k + 1) * 128], identity[0:N, 0:N]
                )
                nc.vector.tensor_copy(kt_sb[:, k * N:(k + 1) * N], tp[:, :])

            # --- Q last: per-agent psum groups, transpose as soon as ready ---
            qt_sb = spool.tile([128, KC * L], BF16)
            for pair in range(4):            # stream 2 agents per DMA chunk
                wtb = stream_chunk(wq, 2 * pair, 2)
                for nl in range(2):
                    n = 2 * pair + nl
                    qp = psum_q.tile([1, D], F32, tag="qacc")
                    for k in range(KC):
                        nc.tensor.matmul(
                            qp[:, :],
                            zq_sb[:, k * (L * L) + n * L + n: k * (L * L) + n * L + n + 1],
                            wtb[:, (nl * KC + k) * 512:(nl * KC + k) * 512 + 512],
                            start=(k == 0),
                            stop=(k == KC - 1),
                        )
                    qrow = wb_pool.tile([1, D], F32, tag="qrow")
                    nc.vector.tensor_copy(qrow[:, :], qp[:, :])
                    for k in range(KC):
                        tp = psum_tr.tile([128, N], F32, tag="tr")
                        nc.tensor.transpose(
                            tp[:, 0:1],
                            qrow[:, k * 128:(k + 1) * 128],
                            identity[0:1, 0:1],
                        )
                        nc.vector.tensor_copy(
                            qt_sb[:, k * L + n:k * L + n + 1], tp[:, 0:1]
                        )

            # --- S_T[j, i] = <Q_j, K_i> (pre-scaled via zq); mask; softmax ---
            s_ps = psum_s.tile([L, N], F32)
            for k in range(KC):
                nc.tensor.matmul(
                    s_ps[:, :],
                    qt_sb[:, k * L:(k + 1) * L],
                    kt_sb[:, k * N:(k + 1) * N],
                    start=(k == 0),
                    stop=(k == KC - 1),
                )
            s_sb = spool.tile([L, N], F32)
            nc.vector.tensor_add(s_sb[:, :], s_ps[:, :], mask_sb[:, :])
            nmax_sb = spool.tile([L, 1], F32)
            nc.vector.tensor_reduce(
                nmax_sb[:, :], s_sb[:, :], mybir.AxisListType.X,
                mybir.AluOpType.max, negate=True,
            )
            p_sb = spool.tile([L, N], F32)
            den_sb = spool.tile([L, 1], F32)
            nc.scalar.activation(
                p_sb[:, :],
                s_sb[:, :],
                mybir.ActivationFunctionType.Exp,
                bias=nmax_sb[:, 0:1],
                scale=1.0,
                accum_out=den_sb[:, :],
            )
            rden_sb = spool.tile([L, 1], F32)
            nc.vector.reciprocal(rden_sb[:, :], den_sb[:, :])
            nc.vector.tensor_scalar_mul(p_sb[:, :], p_sb[:, :], rden_sb[:, 0:1])

            # --- messages[j] = sum_i alpha[i, j] * V[i] ---
            a_ps = psum_tr.tile([N, N], F32, tag="tr")
            nc.tensor.transpose(a_ps[:, 0:L], p_sb[:, :], identity[0:L, 0:L])
            a_sb = spool.tile([N, L], BF16)
            nc.vector.tensor_copy(a_sb[:, :], a_ps[:, 0:L])
            m_ps = psum_acc.tile([L, D], F32, tag="acc")
            nc.tensor.matmul(m_ps[:, :], a_sb[:, :], vf_sb[:, :], start=True, stop=True)
            msg_sb = spool.tile([L, D], F32)
            nc.vector.tensor_copy(msg_sb[:, :], m_ps[:, :])
            nc.sync.dma_start(out=out_msg[:, :], in_=msg_sb[:, :])

    nc.compile()
    return nc


def _prep_inputs(z, Wq, Wk, Wv, Wi, bi):
    """Host-side shard + relayout. Returns in_maps for the 8 cores."""
    z = np.asarray(z, np.float32)
    Wq = np.asarray(Wq, np.float32)
    Wk = np.asarray(Wk, np.float32)
    Wv = np.asarray(Wv, np.float32)
    Wi = np.asarray(Wi, np.float32)
    bi = np.asarray(bi, np.float32)
    scale = np.float32(1.0 / np.sqrt(np.float32(D)))

    # Wi.T chunked (bf16): [p, k*512 + zz] = Wi[zz, k*128+p]
    wi_h = np.ascontiguousarray(
        Wi.T.reshape(KC, 128, Z).transpose(1, 0, 2).reshape(128, KC * Z)
    ).astype(NPBF16)

    def wlayout(Wl):
        # [p, n*2048 + k*512 + d] = Wl[n, d, k*128+p]
        return np.ascontiguousarray(
            Wl.reshape(L, D, KC, 128).transpose(3, 0, 2, 1).reshape(128, L * KC * D)
        )

    def zexpand(zl):
        # [p, k*64 + n*8 + c] = zl[n, k*128+p] * (c == n)
        ze = np.zeros((128, KC, L, L), np.float32)
        for n in range(L):
            ze[:, :, n, n] = zl[n].reshape(KC, 128).T
        return np.ascontiguousarray(ze.reshape(128, KC * L * L)).astype(NPBF16)

    in_maps = []
    for c in range(C):
        sl = slice(c * L, (c + 1) * L)
        zl = z[sl]
        maskT = np.zeros((L, N), np.float32)
        for j in range(L):
            maskT[j, c * L + j] = NEG
        in_maps.append({
            "wq": wlayout(Wq[sl]),
            "wk": wlayout(Wk[sl]),
            "wv": wlayout(Wv[sl]),
            "zq": zexpand(zl * scale),
            "ze": zexpand(zl),
            "wi": wi_h,
            "maskT": maskT,
            "bmz": np.ascontiguousarray(bi[None, :] - zl),
        })
    return in_maps


def run(z, Wq, Wk, Wv, Wi, bi, trace=False, **trace_kwargs):
    if "nc" not in _cache:
        _cache["nc"] = _build()
    nc = _cache["nc"]
    in_maps = _prep_inputs(z, Wq, Wk, Wv, Wi, bi)
    res = run_bass_kernel_spmd(
        nc, in_maps, core_ids=list(range(C)), trace=trace, **trace_kwargs
    )
    msgs = np.concatenate([r["out_msg"] for r in res.results], axis=0)
    sse = np.concatenate([r["out_sse"] for r in res.results], axis=0)
    loss = np.float32(sse.sum() / (N * Z))
    return (msgs, loss), res


def kernel(z, Wq, Wk, Wv, Wi, bi):
    (msgs, loss), _ = run(z, Wq, Wk, Wv, Wi, bi)
    return msgs, loss
